# revision 20
# baseline (speedup 1.0000x reference)
"""Trainium2 Bass kernel for nn_BiDirectionalFusionModule.

Computation (B=4, C=256, CK=32, H=W=64, N=4096):
  two DANet-style non-local attentions (d2r: Q from rgb, K/V from depth;
  r2d: swapped), residual with gamma scaling, channel concat, 3x3 conv
  (512->256) + BN(eval) + ReLU.

Sharding: 8 cores = (batch b, image half). Each core computes BOTH attention
directions for its 34-row query slab (32 output rows + 1 halo row each side,
out-of-range rows zero padded) and then the 3x3 conv for its 32 output rows.
No cross-core communication; the host gathers (B,256,64,64) at the end.

Device layout trick: scores are computed transposed, S^T = k^T q with
j (key index) on partitions and i (query index) free, so E^T=exp(S^T) is
directly the moving operand of the apply matmul out = (v^T).T @ E^T, and v^T
comes from a projection matmul with wv^T as moving operand. Zero on-device
transposes. Softmax normalization: column sums of E^T via col-packed
ones-matmuls (partition reduction on the PE), reciprocal + mask on DVE,
broadcast back over partitions with a K=1 matmul.

All matmuls bf16 (1 cycle/row) with fp32 PSUM accumulation. Host pre-folds:
gamma into wv, BN into conv weights/bias, 4x replication into wq/wk (so the
32-row score matmuls can be row-packed 4x with tile_position).
"""

import numpy as np
import ml_dtypes

BF16 = ml_dtypes.bfloat16

B, C, H, W = 4, 256, 64, 64
N = H * W            # 4096 tokens
CK = 32
NI = 34 * 64         # 2176 query positions per core (34 rows incl. halo)
WP = 68              # padded row width: 2 zero cols each side
NPAD = 34 * WP       # 2312
NOUT = 32 * 64       # 2048 output positions per core
NJC = N // 128       # 32 j-chunks
IBLKS = [(0, 512), (512, 512), (1024, 512), (1536, 512), (2048, 128)]

_CACHE = {}
LAST_RESULTS = None


def _build_program():
    import concourse.tile as tile
    from concourse import bacc, mybir

    f32 = mybir.dt.float32
    bf = mybir.dt.bfloat16
    f8 = mybir.dt.float8e4
    Alu = mybir.AluOpType
    Act = mybir.ActivationFunctionType
    DR = mybir.MatmulPerfMode.DoubleRow
    # exp(S - EXP_SHIFT): keeps E=exp(S') inside fp8e4m3 range; softmax
    # normalization cancels the constant exactly.
    EXP_SHIFT = -2.0

    nc = bacc.Bacc("TRN2", debug=False, enable_asserts=False, num_devices=8)

    # ---- DRAM I/O (per-core data, same names on every core) ----
    d_fq34 = [nc.dram_tensor(f"fq34_{d}", (C, NI), bf, kind="ExternalInput").ap()
              for d in range(2)]
    d_fkv = [nc.dram_tensor(f"fkv_{d}", (C, N), bf, kind="ExternalInput").ap()
             for d in range(2)]
    d_attw = [nc.dram_tensor(f"attw_{d}", (128, 1024), bf, kind="ExternalInput").ap()
              for d in range(2)]
    d_scal = nc.dram_tensor("scal", (128, 8), f32, kind="ExternalInput").ap()
    d_convw = nc.dram_tensor("convw", (128, 72 * 128), bf, kind="ExternalInput").ap()
    d_convb = nc.dram_tensor("convb", (128, 2), f32, kind="ExternalInput").ap()
    d_mask = nc.dram_tensor("mask", (1, NI), f32, kind="ExternalInput").ap()
    d_y = nc.dram_tensor("y", (C, NOUT), f32, kind="ExternalOutput").ap()

    with tile.TileContext(nc) as tc:
        with (
            tc.tile_pool(name="consts", bufs=1) as consts,
            tc.tile_pool(name="big", bufs=1) as big,
            tc.tile_pool(name="stream", bufs=6) as stream,
            tc.tile_pool(name="kq", bufs=2) as kqp,
            tc.tile_pool(name="vt", bufs=32) as vtp,
            tc.tile_pool(name="Ep", bufs=16) as Ep,
            tc.tile_pool(name="small", bufs=3) as small,
            tc.tile_pool(name="yp", bufs=1) as yp,
        ):
            # ---- constants / inputs resident in SBUF ----
            attw = []
            for d in range(2):
                t = consts.tile([128, 1024], bf, name=f"attw{d}", tag=f"attw{d}")
                nc.sync.dma_start(t[:], d_attw[d])
                attw.append(t)
            scal = consts.tile([128, 8], f32, name="scal_sb", tag="scal_sb")
            nc.sync.dma_start(scal[:], d_scal)
            mask = consts.tile([1, NI], f32, name="mask_sb", tag="mask_sb")
            nc.sync.dma_start(mask[:], d_mask)
            # conv weights are not needed until the very end — DMA them late
            # (emitted after dir-0 projections) so they don't delay the start.
            convw = consts.tile([128, 72 * 128], bf, name="convw_sb", tag="convw_sb")
            convb = consts.tile([128, 2], f32, name="convb_sb", tag="convb_sb")

            ones32 = consts.tile([128, 32], f8, name="ones32", tag="ones32")
            nc.vector.memset(ones32[:], 1.0)
            sel = consts.tile([128, 1], bf, name="sel4", tag="sel4")
            nc.vector.memset(sel[:], 0.0)
            for g in range(4):
                nc.vector.memset(sel[32 * g:32 * g + 1, :], 1.0)
            onesk1 = consts.tile([1, 128], bf, name="onesk1", tag="onesk1")
            nc.vector.memset(onesk1[:], 1.0)
            expbias = consts.tile([128, 1], f32, name="expbias", tag="expbias")
            nc.vector.memset(expbias[:], EXP_SHIFT)

            # fq34 DMAs are emitted inside the dir loop (just before they're
            # needed) so the start isn't DMA-bound.
            fq34 = [[big.tile([128, NI], bf, name=f"fq34_{d}_{cc}",
                              tag=f"fq34_{d}_{cc}")
                     for cc in range(2)] for d in range(2)]

            enh = []
            for d in range(2):
                row = []
                for cc in range(2):
                    t = big.tile([128, NPAD], bf, name=f"enh_{d}_{cc}",
                                 tag=f"enh_{d}_{cc}")
                    nc.gpsimd.memset(t[:], 0.0)
                    row.append(t)
                enh.append(row)

            y_sb = [yp.tile([128, NOUT], f32, name=f"y{oc}", tag=f"y{oc}")
                    for oc in range(2)]

            # attw column layout: [q4T cc0 | q4T cc1 | k4T cc0 | k4T cc1 |
            #                      wvT cc0 (256) | wvT cc1 (256)]
            def attw_q4(d, cc):
                return attw[d][:, cc * 128:(cc + 1) * 128]

            def attw_k4(d, cc):
                return attw[d][:, 256 + cc * 128: 256 + (cc + 1) * 128]

            def attw_vT(d, cc):
                return attw[d][:, 512 + cc * 256: 512 + (cc + 1) * 256]

            with tc.tile_pool(name="psA", bufs=1, space="PSUM") as psA:
                for d in range(2):
                    # ================= projections =================
                    k4 = kqp.tile([128, N], bf, name=f"k4_{d}", tag="k4")
                    q4 = kqp.tile([128, NI], bf, name=f"q4_{d}", tag="q4")
                    vts = []
                    for n in range(8):  # 512-wide blocks of f_kv
                        s0 = stream.tile([128, 512], bf, name=f"s0_{d}_{n}",
                                         tag="stream")
                        nc.sync.dma_start(s0[:], d_fkv[d][0:128, n * 512:(n + 1) * 512])
                        s1 = stream.tile([128, 512], bf, name=f"s1_{d}_{n}",
                                         tag="stream")
                        nc.sync.dma_start(s1[:], d_fkv[d][128:256, n * 512:(n + 1) * 512])

                        # k4 chunk: (128, 512) = [wk;wk;wk;wk] @ f_kv block
                        kp = psA.tile([128, 512], f32, name=f"kp_{d}_{n}",
                                      tag="conv", bufs=1)
                        nc.tensor.matmul(kp[:], attw_k4(d, 0), s0[:],
                                         start=True, stop=False)
                        nc.tensor.matmul(kp[:], attw_k4(d, 1), s1[:],
                                         start=False, stop=True)
                        nc.vector.tensor_scalar(
                            k4[:, n * 512:(n + 1) * 512], kp[:],
                            scal[:, 4 * d + 1:4 * d + 2], None, Alu.add)

                        # vT chunks: 4 j-chunks in this block, pairs share a bank
                        for half in range(2):
                            vp = psA.tile([128, 512], f32, name=f"vp_{d}_{n}_{half}",
                                          tag="apply", bufs=2)
                            for jj in range(2):
                                jl = half * 2 + jj
                                nc.tensor.matmul(
                                    vp[:, jj * 256:(jj + 1) * 256],
                                    s0[:, jl * 128:(jl + 1) * 128],
                                    attw_vT(d, 0), start=True, stop=False)
                                nc.tensor.matmul(
                                    vp[:, jj * 256:(jj + 1) * 256],
                                    s1[:, jl * 128:(jl + 1) * 128],
                                    attw_vT(d, 1), start=False, stop=True)
                            vt_t = vtp.tile([128, 512], f8,
                                            name=f"vt_{d}_{n}_{half}", tag="vt")
                            nc.vector.tensor_copy(vt_t[:], vp[:])
                            vts.append(vt_t)

                    for cc in range(2):
                        nc.sync.dma_start(fq34[d][cc][:],
                                          d_fq34[d][cc * 128:(cc + 1) * 128, :])
                    # q4: (128, NI) = [wq;wq;wq;wq] @ f_q34
                    for ib, (i0, iw) in enumerate(IBLKS):
                        qp = psA.tile([128, 512], f32, name=f"qp_{d}_{ib}",
                                      tag="conv", bufs=1)
                        nc.tensor.matmul(qp[:, :iw], attw_q4(d, 0),
                                         fq34[d][0][:, i0:i0 + iw],
                                         start=True, stop=False)
                        nc.tensor.matmul(qp[:, :iw], attw_q4(d, 1),
                                         fq34[d][1][:, i0:i0 + iw],
                                         start=False, stop=True)
                        nc.vector.tensor_scalar(
                            q4[:, i0:i0 + iw], qp[:, :iw],
                            scal[:, 4 * d:4 * d + 1], None, Alu.add)

                    if d == 0:
                        nc.sync.dma_start(convw[:], d_convw)
                        nc.sync.dma_start(convb[:], d_convb)

                    # ================= attention i-blocks =================
                    for ib, (i0, iw) in enumerate(IBLKS):
                        cs = psA.tile([128, 512], f32, name=f"cs_{d}_{ib}",
                                      tag="cs", bufs=1)
                        ap_ps = [psA.tile([128, 512], f32, name=f"ap_{d}_{ib}_{cc}",
                                          tag="apply", bufs=2) for cc in range(2)]
                        for p in range(8):
                            # two 2-bank half-packs (bufs=2) so the next pack's
                            # scores can start while this pack's exp runs —
                            # keeps the PE gap-free (and therefore HAM-warm).
                            halves = [
                                psA.tile([128, 1024], f32,
                                         name=f"pk_{d}_{ib}_{p}_{h}",
                                         tag="pack", bufs=2)
                                for h in range(2)
                            ]
                            E = Ep.tile([128, 2048], f8, name=f"E_{d}_{ib}_{p}",
                                        tag="E")
                            for g in range(4):
                                jc = 4 * p + g
                                h, hg = divmod(g, 2)
                                nc.tensor.matmul(
                                    halves[h][:, hg * 512: hg * 512 + iw],
                                    k4[32 * g:32 * g + 32, jc * 128:(jc + 1) * 128],
                                    q4[32 * g:32 * g + 32, i0:i0 + iw],
                                    start=True, stop=True,
                                    tile_position=(32 * g, 0))
                            if iw == 512:
                                for h in range(2):
                                    nc.scalar.activation(
                                        E[:, h * 1024:(h + 1) * 1024],
                                        halves[h][:], Act.Exp, bias=expbias[:, 0:1])
                            else:
                                for g in range(4):
                                    h, hg = divmod(g, 2)
                                    nc.scalar.activation(
                                        E[:, g * 512: g * 512 + iw],
                                        halves[h][:, hg * 512: hg * 512 + iw],
                                        Act.Exp, bias=expbias[:, 0:1])
                            # fp8 DoubleRow apply: one matmul contracts a
                            # jc-PAIR (K=256) — vt pair tiles are already
                            # [ki, pair, c]-major, E packs [ki, jc, i]-major.
                            for gp in range(2):
                                pair = 2 * p + gp
                                vt3 = vts[pair].rearrange("p (t c) -> p t c", t=2)
                                E3 = E[:, gp * 1024:(gp + 1) * 1024].rearrange(
                                    "p (t i) -> p t i", t=2)
                                for cc in range(2):
                                    nc.tensor.matmul(
                                        ap_ps[cc][:, :iw],
                                        vt3[:, :, cc * 128:(cc + 1) * 128],
                                        E3[:, :, :iw],
                                        perf_mode=DR,
                                        start=(pair == 0), stop=(pair == 15),
                                        skip_group_check=True)
                            for g in range(4):
                                nc.tensor.matmul(
                                    cs[32 * g:32 * g + 32, :iw],
                                    ones32[:], E[:, g * 512: g * 512 + iw],
                                    start=(p == 0), stop=(p == 7),
                                    tile_position=(0, 32 * g),
                                    skip_group_check=True)

                        # ---- softmax normalization + residual ----
                        csum = small.tile([128, 512], bf, name=f"csum_{d}_{ib}",
                                          tag="csum", bufs=2)
                        nc.vector.tensor_copy(csum[:, :iw], cs[:, :iw])
                        fold = psA.tile([1, 512], f32, name=f"fold_{d}_{ib}",
                                        tag="cs", bufs=1)
                        nc.tensor.matmul(fold[:1, :iw], sel[:], csum[:, :iw],
                                         start=True, stop=True)
                        rsb = small.tile([1, 512], f32, name=f"rsb_{d}_{ib}",
                                         tag="rsb", bufs=2)
                        nc.vector.reciprocal_approx_fast(rsb[:1, :iw], fold[:1, :iw])
                        msb = small.tile([1, 512], bf, name=f"msb_{d}_{ib}",
                                         tag="msb", bufs=2)
                        nc.vector.tensor_tensor(msb[:1, :iw], rsb[:1, :iw],
                                                mask[:1, i0:i0 + iw], Alu.mult)
                        bc = psA.tile([128, 512], f32, name=f"bc_{d}_{ib}",
                                      tag="cs", bufs=1)
                        nc.tensor.matmul(bc[:, :iw], onesk1[:], msb[:1, :iw],
                                         start=True, stop=True)
                        rec = small.tile([128, 512], f32, name=f"rec_{d}_{ib}",
                                         tag="rec", bufs=2)
                        nc.vector.tensor_copy(rec[:, :iw], bc[:, :iw])

                        nr = iw // 64
                        r0b = i0 // 64
                        for cc in range(2):
                            tmp = small.tile([128, 512], bf, name=f"tmp_{d}_{ib}_{cc}",
                                             tag="tmp", bufs=3)
                            nc.vector.tensor_tensor(tmp[:, :iw], ap_ps[cc][:, :iw],
                                                    rec[:, :iw], Alu.mult)
                            tmp3 = tmp.rearrange("p (r x) -> p r x", x=64)
                            fq3 = fq34[d][cc].rearrange("p (r x) -> p r x", x=64)
                            enh3 = enh[d][cc].rearrange("p (r x) -> p r x", x=WP)
                            nc.vector.scalar_tensor_tensor(
                                enh3[:, r0b:r0b + nr, 2:66],
                                tmp3[:, :nr, :],
                                scal[:, 4 * d + 2 + cc:4 * d + 3 + cc],
                                fq3[:, r0b:r0b + nr, :],
                                Alu.add, Alu.add)

                # ============ 3x3 conv + BN + ReLU ============
                # 8 sequential 1-bank waves, emitted AFTER the attention so
                # the scheduler uses the (earlier-ready) d2r-channel matmuls
                # to fill PE gaps in the ACT-bound attention phase.
                enh3 = [[enh[d][cc].rearrange("p (r x) -> p r x", x=WP)
                         for cc in range(2)] for d in range(2)]
                for oc in range(2):
                    for sp in range(4):
                        cp = psA.tile([128, 512], f32, name=f"cv_{oc}_{sp}",
                                      tag="conv", bufs=1)
                        first = True
                        for cc4 in range(4):
                            d, cc = divmod(cc4, 2)
                            for ky in range(3):
                                for kx in range(3):
                                    tslot = ((ky * 3 + kx) * 4 + cc4) * 2 + oc
                                    wsl = convw[:, tslot * 128:(tslot + 1) * 128]
                                    rhs = enh3[d][cc][:, sp * 8 + ky: sp * 8 + ky + 8,
                                                      kx + 1: kx + 65]
                                    nc.tensor.matmul(
                                        cp[:], wsl, rhs,
                                        start=first,
                                        stop=(cc4 == 3 and ky == 2 and kx == 2),
                                        skip_group_check=True)
                                    first = False
                        nc.scalar.activation(
                            y_sb[oc][:, sp * 512:(sp + 1) * 512],
                            cp[:], Act.Relu,
                            bias=convb[:, oc:oc + 1])
                        nc.sync.dma_start(
                            d_y[oc * 128:(oc + 1) * 128, sp * 512:(sp + 1) * 512],
                            y_sb[oc][:, sp * 512:(sp + 1) * 512])

    nc.compile()
    return nc


def _get_nc():
    if "nc" not in _CACHE:
        _CACHE["nc"] = _build_program()
    return _CACHE["nc"]


def _host_prep(inputs):
    f32 = np.float32
    ii = {k: np.asarray(v, dtype=f32) if np.asarray(v).dtype.kind == "f"
          else np.asarray(v) for k, v in inputs.items()}

    # ---- shared (core-independent) tensors ----
    attw = np.zeros((2, 128, 1024), f32)
    scal = np.zeros((128, 8), f32)
    for d, sfx in enumerate(("d2r", "r2d")):
        wq, bq = ii[f"wq_{sfx}"], ii[f"bq_{sfx}"]
        wk, bk = ii[f"wk_{sfx}"], ii[f"bk_{sfx}"]
        wv, bv = ii[f"wv_{sfx}"], ii[f"bv_{sfx}"]
        g = float(ii[f"gamma_{sfx}"].reshape(-1)[0])
        wq4t = np.tile(wq, (4, 1)).T.astype(f32)   # (256, 128)
        wk4t = np.tile(wk, (4, 1)).T.astype(f32)
        wvt = (g * wv).T.astype(f32)               # (256, 256)
        attw[d][:, 0:128] = wq4t[0:128]
        attw[d][:, 128:256] = wq4t[128:256]
        attw[d][:, 256:384] = wk4t[0:128]
        attw[d][:, 384:512] = wk4t[128:256]
        attw[d][:, 512:768] = wvt[0:128]
        attw[d][:, 768:1024] = wvt[128:256]
        scal[:, 4 * d + 0] = np.tile(bq, 4)
        scal[:, 4 * d + 1] = np.tile(bk, 4)
        scal[:, 4 * d + 2] = g * bv[0:128]
        scal[:, 4 * d + 3] = g * bv[128:256]

    # conv + BN fold
    eps = f32(1e-5)
    inv = (1.0 / np.sqrt(ii["bn_var"] + eps)).astype(f32)
    sc = inv * ii["bn_scale"]
    wf_f = (ii["wf"] * sc[:, None, None, None]).astype(f32)      # (256,512,3,3)
    bf_f = ((ii["bf"] - ii["bn_mean"]) * sc + ii["bn_bias"]).astype(f32)
    convw = np.zeros((128, 72 * 128), f32)
    for ky in range(3):
        for kx in range(3):
            for cc4 in range(4):
                for oc in range(2):
                    t = ((ky * 3 + kx) * 4 + cc4) * 2 + oc
                    blk = wf_f[oc * 128:(oc + 1) * 128,
                               cc4 * 128:(cc4 + 1) * 128, ky, kx]
                    convw[:, t * 128:(t + 1) * 128] = blk.T
    convb = np.stack([bf_f[0:128], bf_f[128:256]], axis=1).astype(f32)  # (128,2)

    shared = {
        "attw_0": attw[0].astype(BF16),
        "attw_1": attw[1].astype(BF16),
        "scal": scal,
        "convw": convw.astype(BF16),
        "convb": convb,
    }

    # ---- per-core tensors ----
    f_rgb = ii["f_rgb"].reshape(B, C, H, W)
    f_depth = ii["f_depth"].reshape(B, C, H, W)

    def make34(img, r0):  # img (C,H,W) -> (C, NI) bf16, rows [r0-1, r0+33)
        out = np.zeros((C, 34, W), f32)
        lo = r0 - 1
        s_lo, s_hi = max(lo, 0), min(r0 + 33, H)
        out[:, s_lo - lo: s_hi - lo, :] = img[:, s_lo:s_hi, :]
        return out.reshape(C, NI).astype(BF16)

    in_maps = []
    for core in range(8):
        b, half = divmod(core, 2)
        r0 = half * 32
        mask = np.ones((1, NI), f32)
        if half == 0:
            mask[0, 0:64] = 0.0
        else:
            mask[0, NI - 64:NI] = 0.0
        m = dict(shared)
        m["fq34_0"] = make34(f_rgb[b], r0)     # d2r: Q/resid on rgb
        m["fq34_1"] = make34(f_depth[b], r0)   # r2d: Q/resid on depth
        m["fkv_0"] = f_depth[b].reshape(C, N).astype(BF16)  # d2r K/V
        m["fkv_1"] = f_rgb[b].reshape(C, N).astype(BF16)    # r2d K/V
        m["mask"] = mask
        in_maps.append(m)
    return in_maps


def kernel(**inputs):
    global LAST_RESULTS
    from concourse import bass_utils

    nc = _get_nc()
    in_maps = _host_prep(inputs)
    res = bass_utils.run_bass_kernel_spmd(nc, in_maps, core_ids=list(range(8)))
    LAST_RESULTS = res

    y = np.zeros((B, C, H, W), np.float32)
    for core in range(8):
        b, half = divmod(core, 2)
        y[b, :, half * 32:(half + 1) * 32, :] = \
            res.results[core]["y"].reshape(C, 32, W)
    return y


# revision 22
# speedup vs baseline: 1.0246x; 1.0246x over previous
"""Trainium2 Bass kernel for nn_BiDirectionalFusionModule.

Computation (B=4, C=256, CK=32, H=W=64, N=4096):
  two DANet-style non-local attentions (d2r: Q from rgb, K/V from depth;
  r2d: swapped), residual with gamma scaling, channel concat, 3x3 conv
  (512->256) + BN(eval) + ReLU.

Sharding: 8 cores = (batch b, image half). Each core computes BOTH attention
directions for its 34-row query slab (32 output rows + 1 halo row each side,
out-of-range rows zero padded) and then the 3x3 conv for its 32 output rows.
No cross-core communication; the host gathers (B,256,64,64) at the end.

Device layout trick: scores are computed transposed, S^T = k^T q with
j (key index) on partitions and i (query index) free, so E^T=exp(S^T) is
directly the moving operand of the apply matmul out = (v^T).T @ E^T, and v^T
comes from a projection matmul with wv^T as moving operand. Zero on-device
transposes. Softmax normalization: column sums of E^T via col-packed
ones-matmuls (partition reduction on the PE), reciprocal + mask on DVE,
broadcast back over partitions with a K=1 matmul.

All matmuls bf16 (1 cycle/row) with fp32 PSUM accumulation. Host pre-folds:
gamma into wv, BN into conv weights/bias, 4x replication into wq/wk (so the
32-row score matmuls can be row-packed 4x with tile_position).
"""

import numpy as np
import ml_dtypes

BF16 = ml_dtypes.bfloat16

B, C, H, W = 4, 256, 64, 64
N = H * W            # 4096 tokens
CK = 32
NI = 34 * 64         # 2176 query positions per core (34 rows incl. halo)
WP = 68              # padded row width: 2 zero cols each side
NPAD = 34 * WP       # 2312
NOUT = 32 * 64       # 2048 output positions per core
NJC = N // 128       # 32 j-chunks
IBLKS = [(0, 512), (512, 512), (1024, 512), (1536, 512), (2048, 128)]

_CACHE = {}
LAST_RESULTS = None


def _build_program():
    import concourse.tile as tile
    from concourse import bacc, mybir

    f32 = mybir.dt.float32
    bf = mybir.dt.bfloat16
    f8 = mybir.dt.float8e4
    Alu = mybir.AluOpType
    Act = mybir.ActivationFunctionType
    DR = mybir.MatmulPerfMode.DoubleRow
    # exp(S - EXP_SHIFT): keeps E=exp(S') inside fp8e4m3 range; softmax
    # normalization cancels the constant exactly.
    EXP_SHIFT = -2.0

    nc = bacc.Bacc("TRN2", debug=False, enable_asserts=False, num_devices=8)

    # ---- DRAM I/O (per-core data, same names on every core) ----
    d_fq34 = [nc.dram_tensor(f"fq34_{d}", (C, NI), bf, kind="ExternalInput").ap()
              for d in range(2)]
    d_fkv = [nc.dram_tensor(f"fkv_{d}", (C, N), bf, kind="ExternalInput").ap()
             for d in range(2)]
    d_attw = [nc.dram_tensor(f"attw_{d}", (128, 1024), bf, kind="ExternalInput").ap()
              for d in range(2)]
    d_scal = nc.dram_tensor("scal", (128, 8), f32, kind="ExternalInput").ap()
    d_convw = nc.dram_tensor("convw", (128, 72 * 128), bf, kind="ExternalInput").ap()
    d_convb = nc.dram_tensor("convb", (128, 2), f32, kind="ExternalInput").ap()
    d_mask = nc.dram_tensor("mask", (1, NI), f32, kind="ExternalInput").ap()
    d_y = nc.dram_tensor("y", (C, NOUT), f32, kind="ExternalOutput").ap()

    with tile.TileContext(nc) as tc:
        with (
            tc.tile_pool(name="consts", bufs=1) as consts,
            tc.tile_pool(name="big", bufs=1) as big,
            tc.tile_pool(name="stream", bufs=6) as stream,
            tc.tile_pool(name="kq", bufs=2) as kqp,
            tc.tile_pool(name="vt", bufs=32) as vtp,
            tc.tile_pool(name="Ep", bufs=16) as Ep,
            tc.tile_pool(name="small", bufs=3) as small,
            tc.tile_pool(name="yp", bufs=1) as yp,
        ):
            # ---- constants / inputs resident in SBUF ----
            attw = []
            for d in range(2):
                t = consts.tile([128, 1024], bf, name=f"attw{d}", tag=f"attw{d}")
                nc.sync.dma_start(t[:], d_attw[d])
                attw.append(t)
            scal = consts.tile([128, 8], f32, name="scal_sb", tag="scal_sb")
            nc.sync.dma_start(scal[:], d_scal)
            mask = consts.tile([1, NI], f32, name="mask_sb", tag="mask_sb")
            nc.sync.dma_start(mask[:], d_mask)
            # conv weights are not needed until the very end — DMA them late
            # (emitted after dir-0 projections) so they don't delay the start.
            convw = consts.tile([128, 72 * 128], bf, name="convw_sb", tag="convw_sb")
            convb = consts.tile([128, 2], f32, name="convb_sb", tag="convb_sb")

            ones32 = consts.tile([128, 32], f8, name="ones32", tag="ones32")
            nc.vector.memset(ones32[:], 1.0)
            sel = consts.tile([128, 1], bf, name="sel4", tag="sel4")
            nc.vector.memset(sel[:], 0.0)
            for g in range(4):
                nc.vector.memset(sel[32 * g:32 * g + 1, :], 1.0)
            onesk1 = consts.tile([1, 128], bf, name="onesk1", tag="onesk1")
            nc.vector.memset(onesk1[:], 1.0)
            expbias = consts.tile([128, 1], f32, name="expbias", tag="expbias")
            nc.vector.memset(expbias[:], EXP_SHIFT)

            # fq34 DMAs are emitted inside the dir loop (just before they're
            # needed) so the start isn't DMA-bound.
            fq34 = [[big.tile([128, NI], bf, name=f"fq34_{d}_{cc}",
                              tag=f"fq34_{d}_{cc}")
                     for cc in range(2)] for d in range(2)]

            enh = []
            for d in range(2):
                row = []
                for cc in range(2):
                    t = big.tile([128, NPAD], bf, name=f"enh_{d}_{cc}",
                                 tag=f"enh_{d}_{cc}")
                    nc.gpsimd.memset(t[:], 0.0)
                    row.append(t)
                enh.append(row)

            y_sb = [yp.tile([128, NOUT], f32, name=f"y{oc}", tag=f"y{oc}")
                    for oc in range(2)]

            # attw column layout: [q4T cc0 | q4T cc1 | k4T cc0 | k4T cc1 |
            #                      wvT cc0 (256) | wvT cc1 (256)]
            def attw_q4(d, cc):
                return attw[d][:, cc * 128:(cc + 1) * 128]

            def attw_k4(d, cc):
                return attw[d][:, 256 + cc * 128: 256 + (cc + 1) * 128]

            def attw_vT(d, cc):
                return attw[d][:, 512 + cc * 256: 512 + (cc + 1) * 256]

            with tc.tile_pool(name="psA", bufs=1, space="PSUM") as psA:
                k4s, q4s, vtss = [], [], []
                # ====== projections for BOTH dirs up front (overlaps the
                # ACT-bound attention of dir 0 with dir 1's projections) ======
                for d in range(2):
                    k4 = kqp.tile([128, N], bf, name=f"k4_{d}", tag="k4")
                    q4 = kqp.tile([128, NI], bf, name=f"q4_{d}", tag="q4")
                    vts = []
                    k4s.append(k4)
                    q4s.append(q4)
                    vtss.append(vts)
                    for cc in range(2):
                        nc.sync.dma_start(fq34[d][cc][:],
                                          d_fq34[d][cc * 128:(cc + 1) * 128, :])
                    # q4: (128, NI) = [wq;wq;wq;wq] @ f_q34
                    for ib, (i0, iw) in enumerate(IBLKS):
                        qp = psA.tile([128, 512], f32, name=f"qp_{d}_{ib}",
                                      tag="conv", bufs=1)
                        nc.tensor.matmul(qp[:, :iw], attw_q4(d, 0),
                                         fq34[d][0][:, i0:i0 + iw],
                                         start=True, stop=False)
                        nc.tensor.matmul(qp[:, :iw], attw_q4(d, 1),
                                         fq34[d][1][:, i0:i0 + iw],
                                         start=False, stop=True)
                        nc.vector.tensor_scalar(
                            q4[:, i0:i0 + iw], qp[:, :iw],
                            scal[:, 4 * d:4 * d + 1], None, Alu.add)
                    for n in range(8):  # 512-wide blocks of f_kv
                        s0 = stream.tile([128, 512], bf, name=f"s0_{d}_{n}",
                                         tag="stream")
                        nc.sync.dma_start(s0[:], d_fkv[d][0:128, n * 512:(n + 1) * 512])
                        s1 = stream.tile([128, 512], bf, name=f"s1_{d}_{n}",
                                         tag="stream")
                        nc.sync.dma_start(s1[:], d_fkv[d][128:256, n * 512:(n + 1) * 512])

                        # k4 chunk: (128, 512) = [wk;wk;wk;wk] @ f_kv block
                        kp = psA.tile([128, 512], f32, name=f"kp_{d}_{n}",
                                      tag="conv", bufs=1)
                        nc.tensor.matmul(kp[:], attw_k4(d, 0), s0[:],
                                         start=True, stop=False)
                        nc.tensor.matmul(kp[:], attw_k4(d, 1), s1[:],
                                         start=False, stop=True)
                        nc.vector.tensor_scalar(
                            k4[:, n * 512:(n + 1) * 512], kp[:],
                            scal[:, 4 * d + 1:4 * d + 2], None, Alu.add)

                        # vT chunks: 4 j-chunks in this block, pairs share a bank
                        for half in range(2):
                            vp = psA.tile([128, 512], f32, name=f"vp_{d}_{n}_{half}",
                                          tag="apply", bufs=2)
                            for jj in range(2):
                                jl = half * 2 + jj
                                nc.tensor.matmul(
                                    vp[:, jj * 256:(jj + 1) * 256],
                                    s0[:, jl * 128:(jl + 1) * 128],
                                    attw_vT(d, 0), start=True, stop=False)
                                nc.tensor.matmul(
                                    vp[:, jj * 256:(jj + 1) * 256],
                                    s1[:, jl * 128:(jl + 1) * 128],
                                    attw_vT(d, 1), start=False, stop=True)
                            vt_t = vtp.tile([128, 512], f8,
                                            name=f"vt_{d}_{n}_{half}", tag="vt")
                            nc.vector.tensor_copy(vt_t[:], vp[:])
                            vts.append(vt_t)

                nc.sync.dma_start(convw[:], d_convw)
                nc.sync.dma_start(convb[:], d_convb)

                # ================= attention i-blocks =================
                for d in range(2):
                    k4, q4, vts = k4s[d], q4s[d], vtss[d]
                    for ib, (i0, iw) in enumerate(IBLKS):
                        cs = psA.tile([128, 512], f32, name=f"cs_{d}_{ib}",
                                      tag="cs", bufs=1)
                        ap_ps = [psA.tile([128, 512], f32, name=f"ap_{d}_{ib}_{cc}",
                                          tag="apply", bufs=2) for cc in range(2)]
                        for p in range(8):
                            # two 2-bank half-packs (bufs=2) so the next pack's
                            # scores can start while this pack's exp runs —
                            # keeps the PE gap-free (and therefore HAM-warm).
                            halves = [
                                psA.tile([128, 1024], f32,
                                         name=f"pk_{d}_{ib}_{p}_{h}",
                                         tag="pack", bufs=2)
                                for h in range(2)
                            ]
                            E = Ep.tile([128, 2048], f8, name=f"E_{d}_{ib}_{p}",
                                        tag="E")
                            for g in range(4):
                                jc = 4 * p + g
                                h, hg = divmod(g, 2)
                                nc.tensor.matmul(
                                    halves[h][:, hg * 512: hg * 512 + iw],
                                    k4[32 * g:32 * g + 32, jc * 128:(jc + 1) * 128],
                                    q4[32 * g:32 * g + 32, i0:i0 + iw],
                                    start=True, stop=True,
                                    tile_position=(32 * g, 0))
                            if iw == 512:
                                for h in range(2):
                                    nc.scalar.activation(
                                        E[:, h * 1024:(h + 1) * 1024],
                                        halves[h][:], Act.Exp, bias=expbias[:, 0:1])
                            else:
                                for g in range(4):
                                    h, hg = divmod(g, 2)
                                    nc.scalar.activation(
                                        E[:, g * 512: g * 512 + iw],
                                        halves[h][:, hg * 512: hg * 512 + iw],
                                        Act.Exp, bias=expbias[:, 0:1])
                            # fp8 DoubleRow apply: one matmul contracts a
                            # jc-PAIR (K=256) — vt pair tiles are already
                            # [ki, pair, c]-major, E packs [ki, jc, i]-major.
                            for gp in range(2):
                                pair = 2 * p + gp
                                vt3 = vts[pair].rearrange("p (t c) -> p t c", t=2)
                                E3 = E[:, gp * 1024:(gp + 1) * 1024].rearrange(
                                    "p (t i) -> p t i", t=2)
                                for cc in range(2):
                                    nc.tensor.matmul(
                                        ap_ps[cc][:, :iw],
                                        vt3[:, :, cc * 128:(cc + 1) * 128],
                                        E3[:, :, :iw],
                                        perf_mode=DR,
                                        start=(pair == 0), stop=(pair == 15),
                                        skip_group_check=True)
                            for g in range(4):
                                nc.tensor.matmul(
                                    cs[32 * g:32 * g + 32, :iw],
                                    ones32[:], E[:, g * 512: g * 512 + iw],
                                    start=(p == 0), stop=(p == 7),
                                    tile_position=(0, 32 * g),
                                    skip_group_check=True)

                        # ---- softmax normalization + residual ----
                        csum = small.tile([128, 512], bf, name=f"csum_{d}_{ib}",
                                          tag="csum", bufs=2)
                        nc.vector.tensor_copy(csum[:, :iw], cs[:, :iw])
                        fold = psA.tile([1, 512], f32, name=f"fold_{d}_{ib}",
                                        tag="cs", bufs=1)
                        nc.tensor.matmul(fold[:1, :iw], sel[:], csum[:, :iw],
                                         start=True, stop=True)
                        rsb = small.tile([1, 512], f32, name=f"rsb_{d}_{ib}",
                                         tag="rsb", bufs=2)
                        nc.vector.reciprocal_approx_fast(rsb[:1, :iw], fold[:1, :iw])
                        msb = small.tile([1, 512], bf, name=f"msb_{d}_{ib}",
                                         tag="msb", bufs=2)
                        nc.vector.tensor_tensor(msb[:1, :iw], rsb[:1, :iw],
                                                mask[:1, i0:i0 + iw], Alu.mult)
                        bc = psA.tile([128, 512], f32, name=f"bc_{d}_{ib}",
                                      tag="cs", bufs=1)
                        nc.tensor.matmul(bc[:, :iw], onesk1[:], msb[:1, :iw],
                                         start=True, stop=True)
                        rec = small.tile([128, 512], f32, name=f"rec_{d}_{ib}",
                                         tag="rec", bufs=2)
                        nc.vector.tensor_copy(rec[:, :iw], bc[:, :iw])

                        nr = iw // 64
                        r0b = i0 // 64
                        for cc in range(2):
                            tmp = small.tile([128, 512], bf, name=f"tmp_{d}_{ib}_{cc}",
                                             tag="tmp", bufs=3)
                            nc.vector.tensor_tensor(tmp[:, :iw], ap_ps[cc][:, :iw],
                                                    rec[:, :iw], Alu.mult)
                            tmp3 = tmp.rearrange("p (r x) -> p r x", x=64)
                            fq3 = fq34[d][cc].rearrange("p (r x) -> p r x", x=64)
                            enh3 = enh[d][cc].rearrange("p (r x) -> p r x", x=WP)
                            nc.vector.scalar_tensor_tensor(
                                enh3[:, r0b:r0b + nr, 2:66],
                                tmp3[:, :nr, :],
                                scal[:, 4 * d + 2 + cc:4 * d + 3 + cc],
                                fq3[:, r0b:r0b + nr, :],
                                Alu.add, Alu.add)

                # ============ 3x3 conv + BN + ReLU ============
                # 8 sequential 1-bank waves, emitted AFTER the attention so
                # the scheduler uses the (earlier-ready) d2r-channel matmuls
                # to fill PE gaps in the ACT-bound attention phase.
                enh3 = [[enh[d][cc].rearrange("p (r x) -> p r x", x=WP)
                         for cc in range(2)] for d in range(2)]
                for oc in range(2):
                    for sp in range(4):
                        cp = psA.tile([128, 512], f32, name=f"cv_{oc}_{sp}",
                                      tag="conv", bufs=1)
                        first = True
                        for cc4 in range(4):
                            d, cc = divmod(cc4, 2)
                            for ky in range(3):
                                for kx in range(3):
                                    tslot = ((ky * 3 + kx) * 4 + cc4) * 2 + oc
                                    wsl = convw[:, tslot * 128:(tslot + 1) * 128]
                                    rhs = enh3[d][cc][:, sp * 8 + ky: sp * 8 + ky + 8,
                                                      kx + 1: kx + 65]
                                    nc.tensor.matmul(
                                        cp[:], wsl, rhs,
                                        start=first,
                                        stop=(cc4 == 3 and ky == 2 and kx == 2),
                                        skip_group_check=True)
                                    first = False
                        nc.scalar.activation(
                            y_sb[oc][:, sp * 512:(sp + 1) * 512],
                            cp[:], Act.Relu,
                            bias=convb[:, oc:oc + 1])
                        nc.sync.dma_start(
                            d_y[oc * 128:(oc + 1) * 128, sp * 512:(sp + 1) * 512],
                            y_sb[oc][:, sp * 512:(sp + 1) * 512])

    nc.compile()
    return nc


def _get_nc():
    if "nc" not in _CACHE:
        _CACHE["nc"] = _build_program()
    return _CACHE["nc"]


def _host_prep(inputs):
    f32 = np.float32
    ii = {k: np.asarray(v, dtype=f32) if np.asarray(v).dtype.kind == "f"
          else np.asarray(v) for k, v in inputs.items()}

    # ---- shared (core-independent) tensors ----
    attw = np.zeros((2, 128, 1024), f32)
    scal = np.zeros((128, 8), f32)
    for d, sfx in enumerate(("d2r", "r2d")):
        wq, bq = ii[f"wq_{sfx}"], ii[f"bq_{sfx}"]
        wk, bk = ii[f"wk_{sfx}"], ii[f"bk_{sfx}"]
        wv, bv = ii[f"wv_{sfx}"], ii[f"bv_{sfx}"]
        g = float(ii[f"gamma_{sfx}"].reshape(-1)[0])
        wq4t = np.tile(wq, (4, 1)).T.astype(f32)   # (256, 128)
        wk4t = np.tile(wk, (4, 1)).T.astype(f32)
        wvt = (g * wv).T.astype(f32)               # (256, 256)
        attw[d][:, 0:128] = wq4t[0:128]
        attw[d][:, 128:256] = wq4t[128:256]
        attw[d][:, 256:384] = wk4t[0:128]
        attw[d][:, 384:512] = wk4t[128:256]
        attw[d][:, 512:768] = wvt[0:128]
        attw[d][:, 768:1024] = wvt[128:256]
        scal[:, 4 * d + 0] = np.tile(bq, 4)
        scal[:, 4 * d + 1] = np.tile(bk, 4)
        scal[:, 4 * d + 2] = g * bv[0:128]
        scal[:, 4 * d + 3] = g * bv[128:256]

    # conv + BN fold
    eps = f32(1e-5)
    inv = (1.0 / np.sqrt(ii["bn_var"] + eps)).astype(f32)
    sc = inv * ii["bn_scale"]
    wf_f = (ii["wf"] * sc[:, None, None, None]).astype(f32)      # (256,512,3,3)
    bf_f = ((ii["bf"] - ii["bn_mean"]) * sc + ii["bn_bias"]).astype(f32)
    convw = np.zeros((128, 72 * 128), f32)
    for ky in range(3):
        for kx in range(3):
            for cc4 in range(4):
                for oc in range(2):
                    t = ((ky * 3 + kx) * 4 + cc4) * 2 + oc
                    blk = wf_f[oc * 128:(oc + 1) * 128,
                               cc4 * 128:(cc4 + 1) * 128, ky, kx]
                    convw[:, t * 128:(t + 1) * 128] = blk.T
    convb = np.stack([bf_f[0:128], bf_f[128:256]], axis=1).astype(f32)  # (128,2)

    shared = {
        "attw_0": attw[0].astype(BF16),
        "attw_1": attw[1].astype(BF16),
        "scal": scal,
        "convw": convw.astype(BF16),
        "convb": convb,
    }

    # ---- per-core tensors ----
    f_rgb = ii["f_rgb"].reshape(B, C, H, W)
    f_depth = ii["f_depth"].reshape(B, C, H, W)

    def make34(img, r0):  # img (C,H,W) -> (C, NI) bf16, rows [r0-1, r0+33)
        out = np.zeros((C, 34, W), f32)
        lo = r0 - 1
        s_lo, s_hi = max(lo, 0), min(r0 + 33, H)
        out[:, s_lo - lo: s_hi - lo, :] = img[:, s_lo:s_hi, :]
        return out.reshape(C, NI).astype(BF16)

    in_maps = []
    for core in range(8):
        b, half = divmod(core, 2)
        r0 = half * 32
        mask = np.ones((1, NI), f32)
        if half == 0:
            mask[0, 0:64] = 0.0
        else:
            mask[0, NI - 64:NI] = 0.0
        m = dict(shared)
        m["fq34_0"] = make34(f_rgb[b], r0)     # d2r: Q/resid on rgb
        m["fq34_1"] = make34(f_depth[b], r0)   # r2d: Q/resid on depth
        m["fkv_0"] = f_depth[b].reshape(C, N).astype(BF16)  # d2r K/V
        m["fkv_1"] = f_rgb[b].reshape(C, N).astype(BF16)    # r2d K/V
        m["mask"] = mask
        in_maps.append(m)
    return in_maps


def kernel(**inputs):
    global LAST_RESULTS
    from concourse import bass_utils

    nc = _get_nc()
    in_maps = _host_prep(inputs)
    res = bass_utils.run_bass_kernel_spmd(nc, in_maps, core_ids=list(range(8)))
    LAST_RESULTS = res

    y = np.zeros((B, C, H, W), np.float32)
    for core in range(8):
        b, half = divmod(core, 2)
        y[b, :, half * 32:(half + 1) * 32, :] = \
            res.results[core]["y"].reshape(C, 32, W)
    return y


# revision 27
# speedup vs baseline: 1.0252x; 1.0006x over previous
"""Trainium2 Bass kernel for nn_BiDirectionalFusionModule.

Computation (B=4, C=256, CK=32, H=W=64, N=4096):
  two DANet-style non-local attentions (d2r: Q from rgb, K/V from depth;
  r2d: swapped), residual with gamma scaling, channel concat, 3x3 conv
  (512->256) + BN(eval) + ReLU.

Sharding: 8 cores = (batch b, image half). Each core computes BOTH attention
directions for its 34-row query slab (32 output rows + 1 halo row each side,
out-of-range rows zero padded) and then the 3x3 conv for its 32 output rows.
No cross-core communication; the host gathers (B,256,64,64) at the end.

Device layout trick: scores are computed transposed, S^T = k^T q with
j (key index) on partitions and i (query index) free, so E^T=exp(S^T) is
directly the moving operand of the apply matmul out = (v^T).T @ E^T, and v^T
comes from a projection matmul with wv^T as moving operand. Zero on-device
transposes. Softmax normalization: column sums of E^T via col-packed
ones-matmuls (partition reduction on the PE), reciprocal + mask on DVE,
broadcast back over partitions with a K=1 matmul.

All matmuls bf16 (1 cycle/row) with fp32 PSUM accumulation. Host pre-folds:
gamma into wv, BN into conv weights/bias, 4x replication into wq/wk (so the
32-row score matmuls can be row-packed 4x with tile_position).
"""

import numpy as np
import ml_dtypes

BF16 = ml_dtypes.bfloat16

B, C, H, W = 4, 256, 64, 64
N = H * W            # 4096 tokens
CK = 32
NI = 34 * 64         # 2176 query positions per core (34 rows incl. halo)
WP = 68              # padded row width: 2 zero cols each side
NPAD = 34 * WP       # 2312
NOUT = 32 * 64       # 2048 output positions per core
NJC = N // 128       # 32 j-chunks
IBLKS = [(0, 512), (512, 512), (1024, 512), (1536, 512), (2048, 128)]

_CACHE = {}
LAST_RESULTS = None


def _build_program():
    import concourse.tile as tile
    from concourse import bacc, mybir

    f32 = mybir.dt.float32
    bf = mybir.dt.bfloat16
    f8 = mybir.dt.float8e4
    Alu = mybir.AluOpType
    Act = mybir.ActivationFunctionType
    DR = mybir.MatmulPerfMode.DoubleRow
    # exp(S - EXP_SHIFT): keeps E=exp(S') inside fp8e4m3 range; softmax
    # normalization cancels the constant exactly.
    EXP_SHIFT = -2.0

    nc = bacc.Bacc("TRN2", debug=False, enable_asserts=False, num_devices=8)

    # ---- DRAM I/O (per-core data, same names on every core) ----
    d_fq34 = [nc.dram_tensor(f"fq34_{d}", (C, NI), bf, kind="ExternalInput").ap()
              for d in range(2)]
    d_fkv = [nc.dram_tensor(f"fkv_{d}", (C, N), bf, kind="ExternalInput").ap()
             for d in range(2)]
    d_attw = [nc.dram_tensor(f"attw_{d}", (128, 1024), bf, kind="ExternalInput").ap()
              for d in range(2)]
    d_scal = nc.dram_tensor("scal", (128, 8), f32, kind="ExternalInput").ap()
    d_convw = nc.dram_tensor("convw", (128, 72 * 128), bf, kind="ExternalInput").ap()
    d_convb = nc.dram_tensor("convb", (128, 2), f32, kind="ExternalInput").ap()
    d_mask = nc.dram_tensor("mask", (1, NI), f32, kind="ExternalInput").ap()
    d_y = nc.dram_tensor("y", (C, NOUT), f32, kind="ExternalOutput").ap()

    with tile.TileContext(nc) as tc:
        with (
            tc.tile_pool(name="consts", bufs=1) as consts,
            tc.tile_pool(name="big", bufs=1) as big,
            tc.tile_pool(name="stream", bufs=6) as stream,
            tc.tile_pool(name="kq", bufs=2) as kqp,
            tc.tile_pool(name="vt", bufs=32) as vtp,
            tc.tile_pool(name="Ep", bufs=16) as Ep,
            tc.tile_pool(name="small", bufs=3) as small,
            tc.tile_pool(name="yp", bufs=1) as yp,
        ):
            # ---- constants / inputs resident in SBUF ----
            attw = []
            for d in range(2):
                t = consts.tile([128, 1024], bf, name=f"attw{d}", tag=f"attw{d}")
                nc.sync.dma_start(t[:], d_attw[d])
                attw.append(t)
            scal = consts.tile([128, 8], f32, name="scal_sb", tag="scal_sb")
            nc.sync.dma_start(scal[:], d_scal)
            mask = consts.tile([1, NI], f32, name="mask_sb", tag="mask_sb")
            nc.sync.dma_start(mask[:], d_mask)
            # conv weights are not needed until the very end — DMA them late
            # (emitted after dir-0 projections) so they don't delay the start.
            convw = consts.tile([128, 72 * 128], bf, name="convw_sb", tag="convw_sb")
            convb = consts.tile([128, 2], f32, name="convb_sb", tag="convb_sb")

            ones128 = consts.tile([128, 128], f8, name="ones128", tag="ones128")
            nc.vector.memset(ones128[:], 1.0)
            sel = consts.tile([128, 1], bf, name="sel4", tag="sel4")
            nc.vector.memset(sel[:], 0.0)
            for g in range(4):
                nc.vector.memset(sel[32 * g:32 * g + 1, :], 1.0)
            onesk1 = consts.tile([1, 128], bf, name="onesk1", tag="onesk1")
            nc.vector.memset(onesk1[:], 1.0)
            expbias = consts.tile([128, 1], f32, name="expbias", tag="expbias")
            nc.vector.memset(expbias[:], EXP_SHIFT)

            # fq34 DMAs are emitted inside the dir loop (just before they're
            # needed) so the start isn't DMA-bound.
            fq34 = [[big.tile([128, NI], bf, name=f"fq34_{d}_{cc}",
                              tag=f"fq34_{d}_{cc}")
                     for cc in range(2)] for d in range(2)]

            enh = []
            for d in range(2):
                row = []
                for cc in range(2):
                    t = big.tile([128, NPAD], bf, name=f"enh_{d}_{cc}",
                                 tag=f"enh_{d}_{cc}")
                    nc.gpsimd.memset(t[:], 0.0)
                    row.append(t)
                enh.append(row)

            y_sb = [yp.tile([128, NOUT], f32, name=f"y{oc}", tag=f"y{oc}")
                    for oc in range(2)]

            # attw column layout: [q4T cc0 | q4T cc1 | k4T cc0 | k4T cc1 |
            #                      wvT cc0 (256) | wvT cc1 (256)]
            def attw_q4(d, cc):
                return attw[d][:, cc * 128:(cc + 1) * 128]

            def attw_k4(d, cc):
                return attw[d][:, 256 + cc * 128: 256 + (cc + 1) * 128]

            def attw_vT(d, cc):
                return attw[d][:, 512 + cc * 256: 512 + (cc + 1) * 256]

            with tc.tile_pool(name="psA", bufs=1, space="PSUM") as psA:
                k4s, q4s, vtss = [], [], []
                # ====== projections for BOTH dirs up front (overlaps the
                # ACT-bound attention of dir 0 with dir 1's projections) ======
                for d in range(2):
                    k4 = kqp.tile([128, N], bf, name=f"k4_{d}", tag="k4")
                    q4 = kqp.tile([128, NI], bf, name=f"q4_{d}", tag="q4")
                    vts = []
                    k4s.append(k4)
                    q4s.append(q4)
                    vtss.append(vts)
                    for cc in range(2):
                        nc.sync.dma_start(fq34[d][cc][:],
                                          d_fq34[d][cc * 128:(cc + 1) * 128, :])
                    # q4: (128, NI) = [wq;wq;wq;wq] @ f_q34
                    for ib, (i0, iw) in enumerate(IBLKS):
                        qp = psA.tile([128, 512], f32, name=f"qp_{d}_{ib}",
                                      tag="conv", bufs=1)
                        nc.tensor.matmul(qp[:, :iw], attw_q4(d, 0),
                                         fq34[d][0][:, i0:i0 + iw],
                                         start=True, stop=False)
                        nc.tensor.matmul(qp[:, :iw], attw_q4(d, 1),
                                         fq34[d][1][:, i0:i0 + iw],
                                         start=False, stop=True)
                        nc.vector.tensor_scalar(
                            q4[:, i0:i0 + iw], qp[:, :iw],
                            scal[:, 4 * d:4 * d + 1], None, Alu.add)
                    for n in range(8):  # 512-wide blocks of f_kv
                        s0 = stream.tile([128, 512], bf, name=f"s0_{d}_{n}",
                                         tag="stream")
                        nc.sync.dma_start(s0[:], d_fkv[d][0:128, n * 512:(n + 1) * 512])
                        s1 = stream.tile([128, 512], bf, name=f"s1_{d}_{n}",
                                         tag="stream")
                        nc.sync.dma_start(s1[:], d_fkv[d][128:256, n * 512:(n + 1) * 512])

                        # k4 chunk: (128, 512) = [wk;wk;wk;wk] @ f_kv block
                        kp = psA.tile([128, 512], f32, name=f"kp_{d}_{n}",
                                      tag="conv", bufs=1)
                        nc.tensor.matmul(kp[:], attw_k4(d, 0), s0[:],
                                         start=True, stop=False)
                        nc.tensor.matmul(kp[:], attw_k4(d, 1), s1[:],
                                         start=False, stop=True)
                        nc.vector.tensor_scalar(
                            k4[:, n * 512:(n + 1) * 512], kp[:],
                            scal[:, 4 * d + 1:4 * d + 2], None, Alu.add)

                        # vT chunks: 4 j-chunks in this block, pairs share a bank
                        for half in range(2):
                            vp = psA.tile([128, 512], f32, name=f"vp_{d}_{n}_{half}",
                                          tag="apply", bufs=2)
                            for jj in range(2):
                                jl = half * 2 + jj
                                nc.tensor.matmul(
                                    vp[:, jj * 256:(jj + 1) * 256],
                                    s0[:, jl * 128:(jl + 1) * 128],
                                    attw_vT(d, 0), start=True, stop=False)
                                nc.tensor.matmul(
                                    vp[:, jj * 256:(jj + 1) * 256],
                                    s1[:, jl * 128:(jl + 1) * 128],
                                    attw_vT(d, 1), start=False, stop=True)
                            vt_t = vtp.tile([128, 512], f8,
                                            name=f"vt_{d}_{n}_{half}", tag="vt")
                            nc.vector.tensor_copy(vt_t[:], vp[:])
                            vts.append(vt_t)

                nc.sync.dma_start(convw[:], d_convw)
                nc.sync.dma_start(convb[:], d_convb)

                # ================= attention i-blocks =================
                for d in range(2):
                    k4, q4, vts = k4s[d], q4s[d], vtss[d]
                    for ib, (i0, iw) in enumerate(IBLKS):
                        cs = psA.tile([128, 512], f32, name=f"cs_{d}_{ib}",
                                      tag="cs", bufs=1)
                        ap_ps = [psA.tile([128, 512], f32, name=f"ap_{d}_{ib}_{cc}",
                                          tag="apply", bufs=2) for cc in range(2)]
                        for p in range(8):
                            # two 2-bank half-packs (bufs=2) so the next pack's
                            # scores can start while this pack's exp runs —
                            # keeps the PE gap-free (and therefore HAM-warm).
                            halves = [
                                psA.tile([128, 1024], f32,
                                         name=f"pk_{d}_{ib}_{p}_{h}",
                                         tag="pack", bufs=2)
                                for h in range(2)
                            ]
                            E = Ep.tile([128, 2048], f8, name=f"E_{d}_{ib}_{p}",
                                        tag="E")
                            for g in range(4):
                                jc = 4 * p + g
                                h, hg = divmod(g, 2)
                                nc.tensor.matmul(
                                    halves[h][:, hg * 512: hg * 512 + iw],
                                    k4[32 * g:32 * g + 32, jc * 128:(jc + 1) * 128],
                                    q4[32 * g:32 * g + 32, i0:i0 + iw],
                                    start=True, stop=True,
                                    tile_position=(32 * g, 0))
                            if iw == 512:
                                for h in range(2):
                                    nc.scalar.activation(
                                        E[:, h * 1024:(h + 1) * 1024],
                                        halves[h][:], Act.Exp, bias=expbias[:, 0:1])
                            else:
                                for g in range(4):
                                    h, hg = divmod(g, 2)
                                    nc.scalar.activation(
                                        E[:, g * 512: g * 512 + iw],
                                        halves[h][:, hg * 512: hg * 512 + iw],
                                        Act.Exp, bias=expbias[:, 0:1])
                            # fp8 DoubleRow apply: one matmul contracts a
                            # jc-PAIR (K=256) — vt pair tiles are already
                            # [ki, pair, c]-major, E packs [ki, jc, i]-major.
                            for gp in range(2):
                                pair = 2 * p + gp
                                vt3 = vts[pair].rearrange("p (t c) -> p t c", t=2)
                                E3 = E[:, gp * 1024:(gp + 1) * 1024].rearrange(
                                    "p (t i) -> p t i", t=2)
                                for cc in range(2):
                                    nc.tensor.matmul(
                                        ap_ps[cc][:, :iw],
                                        vt3[:, :, cc * 128:(cc + 1) * 128],
                                        E3[:, :, :iw],
                                        perf_mode=DR,
                                        start=(pair == 0), stop=(pair == 15),
                                        skip_group_check=True)
                            for g in range(4):
                                nc.tensor.matmul(
                                    cs[32 * g:32 * g + 32, :iw],
                                    ones128[:, 0:32], E[:, g * 512: g * 512 + iw],
                                    start=(p == 0), stop=(p == 7),
                                    tile_position=(0, 32 * g),
                                    skip_group_check=True)

                        # ---- softmax normalization + residual ----
                        csum = small.tile([128, 512], bf, name=f"csum_{d}_{ib}",
                                          tag="csum", bufs=2)
                        nc.vector.tensor_copy(csum[:, :iw], cs[:, :iw])
                        fold = psA.tile([1, 512], f32, name=f"fold_{d}_{ib}",
                                        tag="cs", bufs=1)
                        nc.tensor.matmul(fold[:1, :iw], sel[:], csum[:, :iw],
                                         start=True, stop=True)
                        rsb = small.tile([1, 512], f32, name=f"rsb_{d}_{ib}",
                                         tag="rsb", bufs=2)
                        nc.vector.reciprocal_approx_fast(rsb[:1, :iw], fold[:1, :iw])
                        msb = small.tile([1, 512], bf, name=f"msb_{d}_{ib}",
                                         tag="msb", bufs=2)
                        nc.vector.tensor_tensor(msb[:1, :iw], rsb[:1, :iw],
                                                mask[:1, i0:i0 + iw], Alu.mult)
                        bc = psA.tile([128, 512], f32, name=f"bc_{d}_{ib}",
                                      tag="cs", bufs=1)
                        nc.tensor.matmul(bc[:, :iw], onesk1[:], msb[:1, :iw],
                                         start=True, stop=True)
                        rec = small.tile([128, 512], f32, name=f"rec_{d}_{ib}",
                                         tag="rec", bufs=2)
                        nc.vector.tensor_copy(rec[:, :iw], bc[:, :iw])

                        nr = iw // 64
                        r0b = i0 // 64
                        for cc in range(2):
                            tmp = small.tile([128, 512], bf, name=f"tmp_{d}_{ib}_{cc}",
                                             tag="tmp", bufs=3)
                            nc.vector.tensor_tensor(tmp[:, :iw], ap_ps[cc][:, :iw],
                                                    rec[:, :iw], Alu.mult)
                            tmp3 = tmp.rearrange("p (r x) -> p r x", x=64)
                            fq3 = fq34[d][cc].rearrange("p (r x) -> p r x", x=64)
                            enh3 = enh[d][cc].rearrange("p (r x) -> p r x", x=WP)
                            nc.vector.scalar_tensor_tensor(
                                enh3[:, r0b:r0b + nr, 2:66],
                                tmp3[:, :nr, :],
                                scal[:, 4 * d + 2 + cc:4 * d + 3 + cc],
                                fq3[:, r0b:r0b + nr, :],
                                Alu.add, Alu.add)

                # ============ 3x3 conv + BN + ReLU ============
                # 8 sequential 1-bank waves, emitted AFTER the attention so
                # the scheduler uses the (earlier-ready) d2r-channel matmuls
                # to fill PE gaps in the ACT-bound attention phase.
                enh3 = [[enh[d][cc].rearrange("p (r x) -> p r x", x=WP)
                         for cc in range(2)] for d in range(2)]
                for oc in range(2):
                    for sp in range(4):
                        # two wave tracks: oc0 rotates the "conv" bank, oc1
                        # the "cs" bank (free after the last normalize) so
                        # post-attention waves pipeline instead of serializing
                        # on a single bank + relu.
                        cp = psA.tile([128, 512], f32, name=f"cv_{oc}_{sp}",
                                      tag=("conv" if oc == 0 else "cs"), bufs=1)
                        first = True
                        for cc4 in range(4):
                            d, cc = divmod(cc4, 2)
                            for ky in range(3):
                                for kx in range(3):
                                    tslot = ((ky * 3 + kx) * 4 + cc4) * 2 + oc
                                    wsl = convw[:, tslot * 128:(tslot + 1) * 128]
                                    rhs = enh3[d][cc][:, sp * 8 + ky: sp * 8 + ky + 8,
                                                      kx + 1: kx + 65]
                                    nc.tensor.matmul(
                                        cp[:], wsl, rhs,
                                        start=first,
                                        stop=(cc4 == 3 and ky == 2 and kx == 2),
                                        skip_group_check=True)
                                    first = False
                        nc.scalar.activation(
                            y_sb[oc][:, sp * 512:(sp + 1) * 512],
                            cp[:], Act.Relu,
                            bias=convb[:, oc:oc + 1])
                        nc.sync.dma_start(
                            d_y[oc * 128:(oc + 1) * 128, sp * 512:(sp + 1) * 512],
                            y_sb[oc][:, sp * 512:(sp + 1) * 512])

    nc.compile()
    return nc


def _get_nc():
    if "nc" not in _CACHE:
        _CACHE["nc"] = _build_program()
    return _CACHE["nc"]


def _host_prep(inputs):
    f32 = np.float32
    ii = {k: np.asarray(v, dtype=f32) if np.asarray(v).dtype.kind == "f"
          else np.asarray(v) for k, v in inputs.items()}

    # ---- shared (core-independent) tensors ----
    attw = np.zeros((2, 128, 1024), f32)
    scal = np.zeros((128, 8), f32)
    for d, sfx in enumerate(("d2r", "r2d")):
        wq, bq = ii[f"wq_{sfx}"], ii[f"bq_{sfx}"]
        wk, bk = ii[f"wk_{sfx}"], ii[f"bk_{sfx}"]
        wv, bv = ii[f"wv_{sfx}"], ii[f"bv_{sfx}"]
        g = float(ii[f"gamma_{sfx}"].reshape(-1)[0])
        wq4t = np.tile(wq, (4, 1)).T.astype(f32)   # (256, 128)
        wk4t = np.tile(wk, (4, 1)).T.astype(f32)
        wvt = (g * wv).T.astype(f32)               # (256, 256)
        attw[d][:, 0:128] = wq4t[0:128]
        attw[d][:, 128:256] = wq4t[128:256]
        attw[d][:, 256:384] = wk4t[0:128]
        attw[d][:, 384:512] = wk4t[128:256]
        attw[d][:, 512:768] = wvt[0:128]
        attw[d][:, 768:1024] = wvt[128:256]
        scal[:, 4 * d + 0] = np.tile(bq, 4)
        scal[:, 4 * d + 1] = np.tile(bk, 4)
        scal[:, 4 * d + 2] = g * bv[0:128]
        scal[:, 4 * d + 3] = g * bv[128:256]

    # conv + BN fold
    eps = f32(1e-5)
    inv = (1.0 / np.sqrt(ii["bn_var"] + eps)).astype(f32)
    sc = inv * ii["bn_scale"]
    wf_f = (ii["wf"] * sc[:, None, None, None]).astype(f32)      # (256,512,3,3)
    bf_f = ((ii["bf"] - ii["bn_mean"]) * sc + ii["bn_bias"]).astype(f32)
    convw = np.zeros((128, 72 * 128), f32)
    for ky in range(3):
        for kx in range(3):
            for cc4 in range(4):
                for oc in range(2):
                    t = ((ky * 3 + kx) * 4 + cc4) * 2 + oc
                    blk = wf_f[oc * 128:(oc + 1) * 128,
                               cc4 * 128:(cc4 + 1) * 128, ky, kx]
                    convw[:, t * 128:(t + 1) * 128] = blk.T
    convb = np.stack([bf_f[0:128], bf_f[128:256]], axis=1).astype(f32)  # (128,2)

    shared = {
        "attw_0": attw[0].astype(BF16),
        "attw_1": attw[1].astype(BF16),
        "scal": scal,
        "convw": convw.astype(BF16),
        "convb": convb,
    }

    # ---- per-core tensors ----
    f_rgb = ii["f_rgb"].reshape(B, C, H, W)
    f_depth = ii["f_depth"].reshape(B, C, H, W)

    def make34(img, r0):  # img (C,H,W) -> (C, NI) bf16, rows [r0-1, r0+33)
        out = np.zeros((C, 34, W), f32)
        lo = r0 - 1
        s_lo, s_hi = max(lo, 0), min(r0 + 33, H)
        out[:, s_lo - lo: s_hi - lo, :] = img[:, s_lo:s_hi, :]
        return out.reshape(C, NI).astype(BF16)

    in_maps = []
    for core in range(8):
        b, half = divmod(core, 2)
        r0 = half * 32
        mask = np.ones((1, NI), f32)
        if half == 0:
            mask[0, 0:64] = 0.0
        else:
            mask[0, NI - 64:NI] = 0.0
        m = dict(shared)
        m["fq34_0"] = make34(f_rgb[b], r0)     # d2r: Q/resid on rgb
        m["fq34_1"] = make34(f_depth[b], r0)   # r2d: Q/resid on depth
        m["fkv_0"] = f_depth[b].reshape(C, N).astype(BF16)  # d2r K/V
        m["fkv_1"] = f_rgb[b].reshape(C, N).astype(BF16)    # r2d K/V
        m["mask"] = mask
        in_maps.append(m)
    return in_maps


def kernel(**inputs):
    global LAST_RESULTS
    from concourse import bass_utils

    nc = _get_nc()
    in_maps = _host_prep(inputs)
    res = bass_utils.run_bass_kernel_spmd(nc, in_maps, core_ids=list(range(8)))
    LAST_RESULTS = res

    y = np.zeros((B, C, H, W), np.float32)
    for core in range(8):
        b, half = divmod(core, 2)
        y[b, :, half * 32:(half + 1) * 32, :] = \
            res.results[core]["y"].reshape(C, 32, W)
    return y
